# revision 12
# baseline (speedup 1.0000x reference)
"""Trainium2 Bass kernel for nn_Attention (dense_transformer).

Sharding: 8 cores = 2 batches x 4 heads; each core computes one (batch, head)
attention independently (head/tensor parallel), QKV weights column-sharded and
the output projection row-sharded per head. Host sums the 4 per-head partial
output projections per batch (row-parallel unshard) and adds the bias.

Per-core dataflow (all-fp16 matmul path):
  x_b [256, 4096] fp16 -> q = scale*Wq_h @ x, k = Wk_h @ x   (PE, [64, 4096])
                          vT[m, d] = (x chunk)^T @ WvT_h      (PE, [128, 64])
  T = k^T q  (scores^T, 128-partition operands, zero-padded d)
  E = exp(T)  ACT-primary with a DVE assist on every 4th key block
      (cols 512:1024 of that tile) via a 5-pass bit-trick exp: Schraudolph
      split into an exact 2^K power and r = 1+frac, then a quadratic in r
      with leading coeff 2^-2 folded into the exponent. The assist keeps the
      ACT stream ahead of the PE so the PE never stalls (p-state!).
  [O; denom] = [v; 1]^T @ E, accumulated over 32 key blocks    (PE fp16)
  U = Wout_h @ O  (unnormalized, fp16 out)                     (PE fp16)
Host: out_b = sum_h U_bh / denom_bh + b_out.
"""

import numpy as np

import concourse.bass as bass
import concourse.tile as tile
from concourse import bacc, mybir
from concourse.bass_utils import run_bass_kernel_spmd

HEADS = 4
DIM_HEAD = 64
SCALE = DIM_HEAD**-0.5
B = 2
C = 256  # input channels
N = 4096  # spatial positions (64*64)
NCH = 1024  # n-chunk (query) size of the main pipeline
NB = N // 128  # number of 128-wide key blocks (32)
F32 = mybir.dt.float32
F16 = mybir.dt.float16
I16 = mybir.dt.int16

BIAS = 0.0  # no exp bias needed: max|O|~4.2e3, max|U|~2.5e3, dnm<1.3e4 in fp16
# DVE bit-trick exp constants:
#   u = int16(s*A16 + B16)  encodes y = s*log2e + 15 in 6.10 fixed point
#   r = bitcast_f16((u & 0x03FF) | 0x3C00) = 1 + frac(y)    (exact)
#   P = bitcast_f16(u & 0x7C00)            = 2^(floor(y)-15)  (exact pow2;
#       the quadratic's 2^-2 leading coeff is pre-folded into B16 as -2048)
#   E = ((r + B1)*r + B0) * P  -- quadratic fit of 2^(r-1)/2^-2
_LOG2E = float(np.log2(np.e))
A16 = 1024.0 * _LOG2E
B16 = 15360.0 - 2048.0 + 0.5  # +0.5: f32->i16 truncates
B1 = 0.9678666917447608
B0 = 1.9999941687718028

# exp engine schedule: ACT primary; DVE does cols DCOL: of every 4th tile
# (gpsimd cannot run these ALU opcodes, so only two engines share exp).
DVE_EXP = frozenset(mb for mb in range(NB - 1) if mb % 4 == 3)
DCOL = 448

_CACHED_NC = None


def _build_nc() -> bass.Bass:
    """Per-core program; identical on all 8 cores (SPMD), data differs."""
    nc = bacc.Bacc(None, target_bir_lowering=False, debug=False)
    ALU = mybir.AluOpType

    x = nc.declare_dram_parameter("x", [C, N], F16, isOutput=False)
    wqk = nc.declare_dram_parameter("wqk", [128, 2, 128], F16, isOutput=False)
    wv = nc.declare_dram_parameter("wv", [128, 2, DIM_HEAD], F16, isOutput=False)
    wo = nc.declare_dram_parameter("wo", [DIM_HEAD, C], F16, isOutput=False)
    u = nc.declare_dram_parameter("u", [C, N], F16, isOutput=True)
    dnm = nc.declare_dram_parameter("dnm", [1, N], F32, isOutput=True)

    with (
        tile.TileContext(nc) as tc,
        tc.tile_pool(name="singles", bufs=1) as singles,
        tc.tile_pool(name="psum", bufs=2, space="PSUM") as psum,
        tc.tile_pool(name="psumO", bufs=2, space="PSUM") as psumO,
        tc.tile_pool(name="esb", bufs=7) as esb,
        tc.tile_pool(name="dsb", bufs=2) as dsb,
        tc.tile_pool(name="osb", bufs=2) as osb,
        tc.tile_pool(name="usb", bufs=2) as usb,
    ):
        x0 = singles.tile([128, N], F16)  # channels 0..127
        x1 = singles.tile([128, N], F16)  # channels 128..255
        wqk_sb = singles.tile([128, 2, 128], F16)
        wv_sb = singles.tile([128, 2, DIM_HEAD], F16)
        wo_sb = singles.tile([DIM_HEAD, C], F16)
        q_sb = singles.tile([128, N], F16)  # rows 64:128 zeroed (K padded; the
        k_sb = singles.tile([128, N], F16)  # 64-row variant ran 40% slower)
        # v'^T blocks: [m-block 128, d 64 | ones] -> out partitions 0..64
        vt_sb = singles.tile([128, NB, DIM_HEAD + 1], F16)

        # x load in 256-col chunks, demand-ordered round-robin on 3 DMA queues
        queues = [nc.sync, nc.gpsimd, nc.scalar]
        jobs = [(wqk_sb[:], wqk[:])]
        XCH = N // 16
        for i in range(16):
            xsl = slice(i * XCH, (i + 1) * XCH)
            jobs.append((x0[:, xsl], x[0:128, xsl]))
            jobs.append((x1[:, xsl], x[128:256, xsl]))
        jobs.append((wv_sb[:], wv[:]))
        jobs.append((wo_sb[:], wo[:]))
        for idx, (dst, srcp) in enumerate(jobs):
            queues[idx % 3].dma_start(dst, srcp)

        ones_t = singles.tile([128, 1], F32)
        nc.vector.memset(ones_t[:], 1.0)
        nc.gpsimd.memset(q_sb[DIM_HEAD:128, :], 0.0)
        nc.gpsimd.memset(k_sb[DIM_HEAD:128, :], 0.0)
        nc.vector.tensor_copy(
            vt_sb[:, :, DIM_HEAD], ones_t[:, 0:1].to_broadcast((128, NB))
        )

        # ---- Phase B: projections (shared psum pool; interleaved emission) ----
        def proj_qk(ch):
            # stacked [q; k] projection: one matmul pair per chunk
            sl = slice(ch * 512, (ch + 1) * 512)
            ps = psum.tile([128, 512], F32, tag="t")
            nc.tensor.matmul(ps[:], wqk_sb[:, 0, :], x0[:, sl], start=True, stop=False)
            nc.tensor.matmul(ps[:], wqk_sb[:, 1, :], x1[:, sl], start=False, stop=True)
            nc.scalar.copy(q_sb[0:DIM_HEAD, sl], ps[0:DIM_HEAD, :])
            nc.vector.tensor_copy(k_sb[0:DIM_HEAD, sl], ps[DIM_HEAD:128, :])

        def proj_v4(g):
            # 4 vT blocks into one psum tile; single batched copy on Pool
            ps = psum.tile([128, 4, DIM_HEAD], F32, tag="t")
            for j in range(4):
                mb = g * 4 + j
                sl = slice(mb * 128, (mb + 1) * 128)
                nc.tensor.matmul(
                    ps[:, j, :], x0[:, sl], wv_sb[:, 0, :], start=True, stop=False
                )
                nc.tensor.matmul(
                    ps[:, j, :], x1[:, sl], wv_sb[:, 1, :], start=False, stop=True
                )
            nc.scalar.copy(vt_sb[:, g * 4 : g * 4 + 4, 0:DIM_HEAD], ps[:])

        # what ci=0 consumes first: q/k chunks 0-1, then the rest interleaved
        proj_qk(0)
        proj_qk(1)
        for g in range(NB // 4):
            proj_v4(g)
            if g >= 2:
                proj_qk(g)

        # ---- Phase C: attention + output projection, n-chunks of NCH ----
        def emit_av(ps_o, e_t, mb):
            for s in range(NCH // 512):
                ssl = slice(s * 512, (s + 1) * 512)
                nc.tensor.matmul(
                    ps_o[0 : DIM_HEAD + 1, ssl],
                    vt_sb[:, mb, :],
                    e_t[:, ssl],
                    start=(mb == 0),
                    stop=(mb == NB - 1),
                )

        def emit_u(o_t, n0):
            for half in range(2):
                osl = slice(half * 128, (half + 1) * 128)
                ps_u = psumO.tile([128, NCH], F32, tag="ps_o")
                for s in range(NCH // 512):
                    ssl = slice(s * 512, (s + 1) * 512)
                    nc.tensor.matmul(
                        ps_u[:, ssl],
                        wo_sb[:, osl],
                        o_t[:, ssl],
                        start=True,
                        stop=True,
                    )
                u_t = usb.tile([128, NCH], F16)
                nc.vector.tensor_copy(u_t[:, 0:512], ps_u[:, 0:512])
                nc.scalar.copy(u_t[:, 512:NCH], ps_u[:, 512:NCH])
                nc.gpsimd.dma_start(u[osl, n0 : n0 + NCH], u_t[:])

        def exp_act(e_t, ps_t, csl):
            nc.scalar.activation(
                e_t[:, csl], ps_t[:, csl], mybir.ActivationFunctionType.Exp
            )

        def bit_exp(v, eng, e_slice, s_slice, w):
            u16 = dsb.tile([128, w], I16, tag=f"u16{eng}")
            rr = dsb.tile([128, w], I16, tag=f"rr{eng}")
            pe = dsb.tile([128, w], I16, tag=f"pe{eng}")
            w1 = dsb.tile([128, w], F16, tag=f"w1{eng}")
            v.tensor_scalar(u16[:], s_slice, A16, B16, ALU.mult, ALU.add)
            v.tensor_scalar(rr[:], u16[:], 0x03FF, 0x3C00, ALU.bitwise_and, ALU.bitwise_or)
            v.tensor_scalar(pe[:], u16[:], 0x7C00, None, ALU.bitwise_and)
            rf = rr[:].bitcast(F16)
            v.scalar_tensor_tensor(w1[:], rf, B1, rf, ALU.add, ALU.mult)
            v.scalar_tensor_tensor(e_slice, w1[:], B0, pe[:].bitcast(F16), ALU.add, ALU.mult)

        def exp_dve(e_t, ps_t, csl):
            bit_exp(nc.vector, "d", e_t[:, csl], ps_t[:, csl], NCH - DCOL)


        pend_u = None  # (o_t, n0) awaiting output projection
        for ci in range(N // NCH):
            n0 = ci * NCH
            ps_o = psumO.tile([128, NCH], F32)
            if pend_u is not None:
                emit_u(*pend_u)
            pend = []  # [(e_t, mb)] awaiting AV matmuls; depth-4 pipeline
            for mb in range(NB):
                msl = slice(mb * 128, (mb + 1) * 128)
                ps_t = psum.tile([128, NCH], F32, tag="t")
                for s in range(NCH // 512):
                    ssl = slice(s * 512, (s + 1) * 512)
                    nc.tensor.matmul(
                        ps_t[:, ssl],
                        k_sb[:, msl],
                        q_sb[:, n0 + s * 512 : n0 + (s + 1) * 512],
                        start=True,
                        stop=True,
                    )
                depth = 4 if mb < NB - 3 else 2
                while len(pend) >= depth:
                    emit_av(ps_o, *pend.pop(0))
                e_t = esb.tile([128, NCH], F16)
                if mb in DVE_EXP:
                    exp_act(e_t, ps_t, slice(0, DCOL))
                    exp_dve(e_t, ps_t, slice(DCOL, NCH))
                else:
                    exp_act(e_t, ps_t, slice(0, NCH))
                pend.append((e_t, mb))
            for p in pend:
                emit_av(ps_o, *p)
            o_t = osb.tile([DIM_HEAD, NCH], F16)
            dnm_t = osb.tile([1, NCH], F32, tag="dnm")
            nc.vector.tensor_copy(o_t[:], ps_o[0:DIM_HEAD, :])
            nc.vector.tensor_copy(dnm_t[:], ps_o[DIM_HEAD : DIM_HEAD + 1, :])
            nc.sync.dma_start(dnm[0:1, n0 : n0 + NCH], dnm_t[:])
            pend_u = (o_t, n0)
        emit_u(*pend_u)

    nc.compile()
    return nc


def _get_nc() -> bass.Bass:
    global _CACHED_NC
    if _CACHED_NC is None:
        _CACHED_NC = _build_nc()
    return _CACHED_NC


def _stripe_kxm(w: np.ndarray, dtype) -> np.ndarray:
    """[256, M] -> [128, 2, M] k-subtile layout (c = t*128 + p)."""
    return np.ascontiguousarray(w.reshape(2, 128, -1).transpose(1, 0, 2)).astype(dtype)


def make_in_maps(x, w_qkv, w_out):
    x2 = np.ascontiguousarray(x.reshape(B, C, N)).astype(np.float16)
    in_maps = []
    for core in range(8):
        b, h = divmod(core, HEADS)
        hs = slice(h * DIM_HEAD, (h + 1) * DIM_HEAD)
        wq_ = (w_qkv[0 * C :][hs, :] * SCALE).T  # [256, 64], scale folded
        wk_ = w_qkv[1 * C :][hs, :].T
        wv_ = w_qkv[2 * C :][hs, :].T
        wo_ = w_out[:, hs].T  # [64, 256]
        wqk_ = np.concatenate([wq_, wk_], axis=1)  # [256, 128]
        in_maps.append(
            {
                "x": x2[b],
                "wqk": _stripe_kxm(wqk_, np.float16),
                "wv": _stripe_kxm(wv_, np.float16),
                "wo": np.ascontiguousarray(wo_, dtype=np.float16),
            }
        )
    return in_maps


def combine(results, b_out):
    out = np.zeros((B, C, N), dtype=np.float32)
    for core in range(8):
        b, _h = divmod(core, HEADS)
        r = results[core]
        out[b] += r["u"].reshape(C, N).astype(np.float32) / r["dnm"].reshape(1, N)
    out += b_out.astype(np.float32)[None, :, None]
    return out.reshape(B, C, 64, 64)


def kernel(x, w_qkv, w_out, b_out, _run_kwargs=None):
    nc = _get_nc()
    in_maps = make_in_maps(np.asarray(x), np.asarray(w_qkv), np.asarray(w_out))
    kw = _run_kwargs or {}
    res = run_bass_kernel_spmd(nc, in_maps, list(range(8)), **kw)
    out = combine(res.results, np.asarray(b_out))
    kernel.last_result = res
    return out


# revision 13
# speedup vs baseline: 1.0079x; 1.0079x over previous
"""Trainium2 Bass kernel for nn_Attention (dense_transformer).

Sharding: 8 cores = 2 batches x 4 heads; each core computes one (batch, head)
attention independently (head/tensor parallel), QKV weights column-sharded and
the output projection row-sharded per head. Host sums the 4 per-head partial
output projections per batch (row-parallel unshard) and adds the bias.

Per-core dataflow (all-fp16 matmul path):
  x_b [256, 4096] fp16 -> q = scale*Wq_h @ x, k = Wk_h @ x   (PE, [64, 4096])
                          vT[m, d] = (x chunk)^T @ WvT_h      (PE, [128, 64])
  T = k^T q  (scores^T, 128-partition operands, zero-padded d)
  E = exp(T)  ACT-primary with a DVE assist on every 4th key block
      (cols 512:1024 of that tile) via a 5-pass bit-trick exp: Schraudolph
      split into an exact 2^K power and r = 1+frac, then a quadratic in r
      with leading coeff 2^-2 folded into the exponent. The assist keeps the
      ACT stream ahead of the PE so the PE never stalls (p-state!).
  [O; denom] = [v; 1]^T @ E, accumulated over 32 key blocks    (PE fp16)
  U = Wout_h @ O  (unnormalized, fp16 out)                     (PE fp16)
Host: out_b = sum_h U_bh / denom_bh + b_out.
"""

import numpy as np

import concourse.bass as bass
import concourse.tile as tile
from concourse import bacc, mybir
from concourse.bass_utils import run_bass_kernel_spmd

HEADS = 4
DIM_HEAD = 64
SCALE = DIM_HEAD**-0.5
B = 2
C = 256  # input channels
N = 4096  # spatial positions (64*64)
NCH = 1024  # n-chunk (query) size of the main pipeline
NB = N // 128  # number of 128-wide key blocks (32)
F32 = mybir.dt.float32
F16 = mybir.dt.float16
I16 = mybir.dt.int16

BIAS = 0.0  # no exp bias needed: max|O|~4.2e3, max|U|~2.5e3, dnm<1.3e4 in fp16
# DVE bit-trick exp constants:
#   u = int16(s*A16 + B16)  encodes y = s*log2e + 15 in 6.10 fixed point
#   r = bitcast_f16((u & 0x03FF) | 0x3C00) = 1 + frac(y)    (exact)
#   P = bitcast_f16(u & 0x7C00)            = 2^(floor(y)-15)  (exact pow2;
#       the quadratic's 2^-2 leading coeff is pre-folded into B16 as -2048)
#   E = ((r + B1)*r + B0) * P  -- quadratic fit of 2^(r-1)/2^-2
_LOG2E = float(np.log2(np.e))
A16 = 1024.0 * _LOG2E
B16 = 15360.0 - 2048.0 + 0.5  # +0.5: f32->i16 truncates
B1 = 0.9678666917447608
B0 = 1.9999941687718028

# exp engine schedule: ACT primary; DVE does cols DCOL: of every 4th tile
# (gpsimd cannot run these ALU opcodes, so only two engines share exp).
DVE_EXP = frozenset(mb for mb in range(NB - 1) if mb % 4 == 3)
DCOL = 512

_CACHED_NC = None


def _build_nc() -> bass.Bass:
    """Per-core program; identical on all 8 cores (SPMD), data differs."""
    nc = bacc.Bacc(None, target_bir_lowering=False, debug=False)
    ALU = mybir.AluOpType

    x = nc.declare_dram_parameter("x", [C, N], F16, isOutput=False)
    wqk = nc.declare_dram_parameter("wqk", [128, 2, 128], F16, isOutput=False)
    wv = nc.declare_dram_parameter("wv", [128, 2, DIM_HEAD], F16, isOutput=False)
    wo = nc.declare_dram_parameter("wo", [DIM_HEAD, C], F16, isOutput=False)
    u = nc.declare_dram_parameter("u", [C, N], F16, isOutput=True)
    dnm = nc.declare_dram_parameter("dnm", [1, N], F32, isOutput=True)

    with (
        tile.TileContext(nc) as tc,
        tc.tile_pool(name="singles", bufs=1) as singles,
        tc.tile_pool(name="psum", bufs=2, space="PSUM") as psum,
        tc.tile_pool(name="psumO", bufs=2, space="PSUM") as psumO,
        tc.tile_pool(name="esb", bufs=7) as esb,
        tc.tile_pool(name="dsb", bufs=2) as dsb,
        tc.tile_pool(name="osb", bufs=2) as osb,
        tc.tile_pool(name="usb", bufs=2) as usb,
    ):
        x0 = singles.tile([128, N], F16)  # channels 0..127
        x1 = singles.tile([128, N], F16)  # channels 128..255
        wqk_sb = singles.tile([128, 2, 128], F16)
        wv_sb = singles.tile([128, 2, DIM_HEAD], F16)
        wo_sb = singles.tile([DIM_HEAD, C], F16)
        q_sb = singles.tile([128, N], F16)  # rows 64:128 zeroed (K padded; the
        k_sb = singles.tile([128, N], F16)  # 64-row variant ran 40% slower)
        # v'^T blocks: [m-block 128, d 64 | ones] -> out partitions 0..64
        vt_sb = singles.tile([128, NB, DIM_HEAD + 1], F16)

        # x load in 256-col chunks, demand-ordered round-robin on 3 DMA queues
        queues = [nc.sync, nc.gpsimd, nc.scalar]
        jobs = [(wqk_sb[:], wqk[:])]
        XCH = N // 16
        for i in range(16):
            xsl = slice(i * XCH, (i + 1) * XCH)
            jobs.append((x0[:, xsl], x[0:128, xsl]))
            jobs.append((x1[:, xsl], x[128:256, xsl]))
        jobs.append((wv_sb[:], wv[:]))
        jobs.append((wo_sb[:], wo[:]))
        for idx, (dst, srcp) in enumerate(jobs):
            queues[idx % 3].dma_start(dst, srcp)

        ones_t = singles.tile([128, 1], F32)
        nc.vector.memset(ones_t[:], 1.0)
        nc.gpsimd.memset(q_sb[DIM_HEAD:128, :], 0.0)
        nc.gpsimd.memset(k_sb[DIM_HEAD:128, :], 0.0)
        nc.vector.tensor_copy(
            vt_sb[:, :, DIM_HEAD], ones_t[:, 0:1].to_broadcast((128, NB))
        )

        # ---- Phase B: projections (shared psum pool; interleaved emission) ----
        def proj_qk(ch):
            # stacked [q; k] projection: one matmul pair per chunk
            sl = slice(ch * 512, (ch + 1) * 512)
            ps = psum.tile([128, 512], F32, tag="t")
            nc.tensor.matmul(ps[:], wqk_sb[:, 0, :], x0[:, sl], start=True, stop=False)
            nc.tensor.matmul(ps[:], wqk_sb[:, 1, :], x1[:, sl], start=False, stop=True)
            nc.scalar.copy(q_sb[0:DIM_HEAD, sl], ps[0:DIM_HEAD, :])
            nc.vector.tensor_copy(k_sb[0:DIM_HEAD, sl], ps[DIM_HEAD:128, :])

        def proj_v4(g):
            # 4 vT blocks into one psum tile; single batched copy on Pool
            ps = psum.tile([128, 4, DIM_HEAD], F32, tag="t")
            for j in range(4):
                mb = g * 4 + j
                sl = slice(mb * 128, (mb + 1) * 128)
                nc.tensor.matmul(
                    ps[:, j, :], x0[:, sl], wv_sb[:, 0, :], start=True, stop=False
                )
                nc.tensor.matmul(
                    ps[:, j, :], x1[:, sl], wv_sb[:, 1, :], start=False, stop=True
                )
            nc.scalar.copy(vt_sb[:, g * 4 : g * 4 + 4, 0:DIM_HEAD], ps[:])

        # what ci=0 consumes first: q/k chunks 0-1, then the rest interleaved
        proj_qk(0)
        proj_qk(1)
        for g in range(NB // 4):
            proj_v4(g)
            if g >= 2:
                proj_qk(g)

        # ---- Phase C: attention + output projection, n-chunks of NCH ----
        def emit_av(ps_o, e_t, mb):
            for s in range(NCH // 512):
                ssl = slice(s * 512, (s + 1) * 512)
                nc.tensor.matmul(
                    ps_o[0 : DIM_HEAD + 1, ssl],
                    vt_sb[:, mb, :],
                    e_t[:, ssl],
                    start=(mb == 0),
                    stop=(mb == NB - 1),
                )

        def emit_u(o_t, n0):
            for half in range(2):
                osl = slice(half * 128, (half + 1) * 128)
                ps_u = psumO.tile([128, NCH], F32, tag="ps_o")
                for s in range(NCH // 512):
                    ssl = slice(s * 512, (s + 1) * 512)
                    nc.tensor.matmul(
                        ps_u[:, ssl],
                        wo_sb[:, osl],
                        o_t[:, ssl],
                        start=True,
                        stop=True,
                    )
                u_t = usb.tile([128, NCH], F16)
                nc.vector.tensor_copy(u_t[:, 0:512], ps_u[:, 0:512])
                nc.scalar.copy(u_t[:, 512:NCH], ps_u[:, 512:NCH])
                nc.gpsimd.dma_start(u[osl, n0 : n0 + NCH], u_t[:])

        def exp_act(e_t, ps_t, csl):
            nc.scalar.activation(
                e_t[:, csl], ps_t[:, csl], mybir.ActivationFunctionType.Exp
            )

        def bit_exp(v, eng, e_slice, s_slice, w):
            u16 = dsb.tile([128, w], I16, tag=f"u16{eng}")
            rr = dsb.tile([128, w], I16, tag=f"rr{eng}")
            pe = dsb.tile([128, w], I16, tag=f"pe{eng}")
            w1 = dsb.tile([128, w], F16, tag=f"w1{eng}")
            v.tensor_scalar(u16[:], s_slice, A16, B16, ALU.mult, ALU.add)
            v.tensor_scalar(rr[:], u16[:], 0x03FF, 0x3C00, ALU.bitwise_and, ALU.bitwise_or)
            v.tensor_scalar(pe[:], u16[:], 0x7C00, None, ALU.bitwise_and)
            rf = rr[:].bitcast(F16)
            v.scalar_tensor_tensor(w1[:], rf, B1, rf, ALU.add, ALU.mult)
            v.scalar_tensor_tensor(e_slice, w1[:], B0, pe[:].bitcast(F16), ALU.add, ALU.mult)

        def exp_dve(e_t, ps_t, csl):
            bit_exp(nc.vector, "d", e_t[:, csl], ps_t[:, csl], NCH - DCOL)


        pend_u = None  # (o_t, n0) awaiting output projection
        for ci in range(N // NCH):
            n0 = ci * NCH
            ps_o = psumO.tile([128, NCH], F32)
            if pend_u is not None:
                emit_u(*pend_u)
            pend = []  # [(e_t, mb)] awaiting AV matmuls; depth-4 pipeline
            for mb in range(NB):
                msl = slice(mb * 128, (mb + 1) * 128)
                ps_t = psum.tile([128, NCH], F32, tag="t")
                for s in range(NCH // 512):
                    ssl = slice(s * 512, (s + 1) * 512)
                    nc.tensor.matmul(
                        ps_t[:, ssl],
                        k_sb[:, msl],
                        q_sb[:, n0 + s * 512 : n0 + (s + 1) * 512],
                        start=True,
                        stop=True,
                    )
                if len(pend) >= 4:
                    emit_av(ps_o, *pend.pop(0))
                e_t = esb.tile([128, NCH], F16)
                if mb in DVE_EXP:
                    exp_act(e_t, ps_t, slice(0, DCOL))
                    exp_dve(e_t, ps_t, slice(DCOL, NCH))
                else:
                    exp_act(e_t, ps_t, slice(0, NCH))
                pend.append((e_t, mb))
            for p in pend:
                emit_av(ps_o, *p)
            o_t = osb.tile([DIM_HEAD, NCH], F16)
            dnm_t = osb.tile([1, NCH], F32, tag="dnm")
            nc.vector.tensor_copy(o_t[:], ps_o[0:DIM_HEAD, :])
            nc.vector.tensor_copy(dnm_t[:], ps_o[DIM_HEAD : DIM_HEAD + 1, :])
            nc.sync.dma_start(dnm[0:1, n0 : n0 + NCH], dnm_t[:])
            pend_u = (o_t, n0)
        emit_u(*pend_u)

    nc.compile()
    return nc


def _get_nc() -> bass.Bass:
    global _CACHED_NC
    if _CACHED_NC is None:
        _CACHED_NC = _build_nc()
    return _CACHED_NC


def _stripe_kxm(w: np.ndarray, dtype) -> np.ndarray:
    """[256, M] -> [128, 2, M] k-subtile layout (c = t*128 + p)."""
    return np.ascontiguousarray(w.reshape(2, 128, -1).transpose(1, 0, 2)).astype(dtype)


def make_in_maps(x, w_qkv, w_out):
    x2 = np.ascontiguousarray(x.reshape(B, C, N)).astype(np.float16)
    in_maps = []
    for core in range(8):
        b, h = divmod(core, HEADS)
        hs = slice(h * DIM_HEAD, (h + 1) * DIM_HEAD)
        wq_ = (w_qkv[0 * C :][hs, :] * SCALE).T  # [256, 64], scale folded
        wk_ = w_qkv[1 * C :][hs, :].T
        wv_ = w_qkv[2 * C :][hs, :].T
        wo_ = w_out[:, hs].T  # [64, 256]
        wqk_ = np.concatenate([wq_, wk_], axis=1)  # [256, 128]
        in_maps.append(
            {
                "x": x2[b],
                "wqk": _stripe_kxm(wqk_, np.float16),
                "wv": _stripe_kxm(wv_, np.float16),
                "wo": np.ascontiguousarray(wo_, dtype=np.float16),
            }
        )
    return in_maps


def combine(results, b_out):
    out = np.zeros((B, C, N), dtype=np.float32)
    for core in range(8):
        b, _h = divmod(core, HEADS)
        r = results[core]
        out[b] += r["u"].reshape(C, N).astype(np.float32) / r["dnm"].reshape(1, N)
    out += b_out.astype(np.float32)[None, :, None]
    return out.reshape(B, C, 64, 64)


def kernel(x, w_qkv, w_out, b_out, _run_kwargs=None):
    nc = _get_nc()
    in_maps = make_in_maps(np.asarray(x), np.asarray(w_qkv), np.asarray(w_out))
    kw = _run_kwargs or {}
    res = run_bass_kernel_spmd(nc, in_maps, list(range(8)), **kw)
    out = combine(res.results, np.asarray(b_out))
    kernel.last_result = res
    return out
